# revision 38
# baseline (speedup 1.0000x reference)
"""Trainium2 Bass kernel for nn_CDEModel (neural CDE, RK4 over cubic-spline control).

Strategy (pure data parallel over batch, 8 cores x 512 rows, v2):
  * Host precomputes G_u [128,127] matrices mapping knot values x -> spline
    derivative dX(u) per interval for the 5 RK4 sample points u.
  * Phase 1: transpose-load action channels to T-layout xT [128L, ch*512b]
    (bf16); encoder inputs (s0T/a0T/t0) are host-transposed and DMA'd direct.
  * Phase 2: dX_u = G_u matmuls (bf16) -> bf16 DRAM scratch, action channels
    only (the t channel has dX == 1 exactly and is folded into the x0 block).
  * Phase 3: encoder z0, stream-STACKED layout z [128, 256]: rows 0:64 =
    latent of batch 0:255, rows 64:128 = latent of batch 256:511.
  * Scan (127 intervals x 2 substeps x 4 RK stages):
      one replicating DMA per interval loads dX broadcast tiles bt
      [128, 5u*4pair*512b] bf16. Per stage: mm1 per stream (fp32r) -> relu
      (ACT, bf16 out) -> 4x pair mm2 (bf16, 512 wide) -> pair mults
      (DVE STT / ACT copy + Pool / ACT copy + DVE bf16) -> per-stream fold
      matmuls with ALPHA-SCALED stationaries (1,2,2,1) accumulating a single
      PSUM k-accumulator [128, 256] per substep. z updates read the
      accumulator directly: 6 DVE ops per substep (RK4 algebra rewritten on
      partial sums).
  * Decoder per interval + PE transposes stage batch-major fp16 output
    obuf [128, L*32]; 4 bulk DMAs at the end. Host just casts to fp32.
"""

import sys

import numpy as np

sys.path.insert(0, "/opt/trn_rl_repo")

# ---- problem constants (hardcoded per contract) ----
B = 4096
L = 128
SD = 32          # state dim
AD = 8           # action dim
LD = 64          # latent dim
HID = 128        # hidden dim
XD = AD + 1      # control channels (t + actions)
NCORES = 8
BC = B // NCORES          # 512 batch rows per core
NS = 2                    # batch streams per core
BS = BC // NS             # 256
NI = L - 1                # 127 intervals
NU = 5                    # u grid {0,.25,.5,.75,1}
UVALS = [0.0, 0.25, 0.5, 0.75, 1.0]

_CACHE = {}
LAST_RESULTS = None


def _host_consts():
    n = L - 2
    M = 4.0 * np.eye(n) + np.eye(n, k=1) + np.eye(n, k=-1)
    Minv = np.linalg.inv(M)
    D2 = np.zeros((n, L))
    for i in range(n):
        D2[i, i], D2[i, i + 1], D2[i, i + 2] = 1.0, -2.0, 1.0
    Pfull = np.zeros((L, L))
    Pfull[1:L - 1, :] = 6.0 * (Minv @ D2)
    S0 = np.eye(L)[: L - 1, :]
    S1 = np.eye(L)[1:, :]
    Delta = np.zeros((L - 1, L))
    for i in range(L - 1):
        Delta[i, i], Delta[i, i + 1] = -1.0, 1.0
    Gt = np.zeros((L, NU * NI), np.float32)
    for ui, u in enumerate(UVALS):
        al = -1.0 / 3.0 + u - u * u / 2.0
        be = -1.0 / 6.0 + u * u / 2.0
        G = Delta + al * (S0 @ Pfull) + be * (S1 @ Pfull)
        Gt[:, ui * NI:(ui + 1) * NI] = G.T.astype(np.float32)
    return Gt


def _perm_w2(f_W2, f_b2):
    """Pair-pack W2: block pt covers x_lo=1+2pt (rows 0:64 of fps) and
    x_hi=2+2pt (rows 64:128). W2x0 is the t-channel (x=0) block."""
    W3 = np.asarray(f_W2, np.float32).reshape(HID, LD, XD)
    W2x0 = np.ascontiguousarray(W3[:, :, 0])                     # [128, 64]
    W2pr = np.empty((HID, 4 * HID), np.float32)
    b2r = np.asarray(f_b2, np.float32).reshape(LD, XD)
    b2T = np.zeros((HID, 4), np.float32)
    for pt in range(4):
        W2pr[:, pt * 128:pt * 128 + 64] = W3[:, :, 1 + 2 * pt]
        W2pr[:, pt * 128 + 64:pt * 128 + 128] = W3[:, :, 2 + 2 * pt]
        b2T[0:64, pt] = b2r[:, 1 + 2 * pt]
        b2T[64:128, pt] = b2r[:, 2 + 2 * pt]
    return W2x0, W2pr, b2T, b2r[:, 0].copy()


def _pad_encw(enc_W):
    out = np.zeros((65, LD), np.float32)
    out[0:40] = np.asarray(enc_W, np.float32)[0:40]
    out[64] = np.asarray(enc_W, np.float32)[40]
    return out


def _build_program():
    import concourse.bacc as bacc
    import concourse.bass as bass
    import concourse.mybir as mybir
    import concourse.tile as tile
    from contextlib import ExitStack

    dt = mybir.dt
    F32 = dt.float32
    F32R = dt.float32r
    BF16 = dt.bfloat16
    F16 = dt.float16
    AF = mybir.ActivationFunctionType
    ALU = mybir.AluOpType

    nc = bacc.Bacc("TRN2", target_bir_lowering=False, debug=False,
                   num_devices=NCORES)

    # ---- DRAM tensors ----
    s0T_d = nc.dram_tensor("s0T", [SD, BC], F32R, kind="ExternalInput").ap()
    a0T_d = nc.dram_tensor("a0T", [AD, BC], F32R, kind="ExternalInput").ap()
    t0_d = nc.dram_tensor("t0", [1, BC], F32R, kind="ExternalInput").ap()
    a_d = nc.dram_tensor("a_in", [BC, L * AD], F16, kind="ExternalInput").ap()
    W1_d = nc.dram_tensor("W1", [LD, HID], F32R, kind="ExternalInput").ap()
    b1_d = nc.dram_tensor("b1", [HID, 1], F32, kind="ExternalInput").ap()
    W2x0a_d = nc.dram_tensor("W2x0a", [HID, LD], F32R, kind="ExternalInput").ap()
    W2x0b_d = nc.dram_tensor("W2x0b", [HID, LD], F32R, kind="ExternalInput").ap()
    W2pr_d = nc.dram_tensor("W2pr", [HID, 4 * HID], F32R, kind="ExternalInput").ap()
    b2T_d = nc.dram_tensor("b2T", [HID, 4], F32, kind="ExternalInput").ap()
    Spa_d = nc.dram_tensor("Spa", [HID, LD], F32R, kind="ExternalInput").ap()
    Spb_d = nc.dram_tensor("Spb", [HID, LD], F32R, kind="ExternalInput").ap()
    encW_d = nc.dram_tensor("encW", [65, LD], F32R, kind="ExternalInput").ap()
    encb2_d = nc.dram_tensor("encb2", [LD, 1], F32, kind="ExternalInput").ap()
    decW_d = nc.dram_tensor("decW", [LD, SD], F32R, kind="ExternalInput").ap()
    decb2_d = nc.dram_tensor("decb2", [SD, 1], F32, kind="ExternalInput").ap()
    Gt_d = nc.dram_tensor("Gt", [L, NU * NI], F32R, kind="ExternalInput").ap()
    idF_d = nc.dram_tensor("idF", [L, L], F32, kind="ExternalInput").ap()
    zpad_d = nc.dram_tensor("zpad", [24, BC], F32R, kind="ExternalInput").ap()

    out_ds = [nc.dram_tensor(f"out{c}", [L, L * SD], F16,
                             kind="ExternalOutput").ap() for c in range(4)]
    # canary F32 output: with only 2-byte ExternalOutputs, the runtime's
    # result fetch scrambles every other element; one fetched F32 tensor
    # keeps the binding sane (empirically verified).
    canary_d = nc.dram_tensor("canary", [LD, BC], F32R,
                              kind="ExternalOutput").ap()
    import os as _os
    _DBG = int(_os.environ.get("K_DBGZ", "0"))
    if _DBG:
        dbgz_d = nc.dram_tensor("dbgz", [LD, BC], F32R,
                                kind="ExternalOutput").ap()
        dbgin_d = nc.dram_tensor("dbgin", [65, BC], F32R,
                                 kind="ExternalOutput").ap()
        dbga_d = nc.dram_tensor("dbga", [128, L * AD], BF16,
                                kind="ExternalOutput").ap()
    # dX scratch: [interval, 8 action channel rows, u, batch] bf16
    dx_d = nc.dram_tensor("dx_scratch", [NI, AD, NU, BC], F32R).ap()

    mmr = nc.tensor.matmul
    STT = nc.vector.scalar_tensor_tensor

    with tile.TileContext(nc, trace_sim=False) as tc, ExitStack() as st:
        # ---------- persistent pools ----------
        wp = st.enter_context(tc.tile_pool(name="weights", bufs=1))

        def wtile(name, dram, shape, dtp):
            t = wp.tile(shape, dtp, tag=name)
            nc.sync.dma_start(t[:], dram)
            return t

        W1_s = wtile("W1", W1_d, [LD, HID], F32R)
        b1_s = wtile("b1", b1_d, [HID, 1], F32)
        W2x0a_s = wtile("W2x0a", W2x0a_d, [HID, LD], F32R)
        W2x0b_s = wtile("W2x0b", W2x0b_d, [HID, LD], F32R)
        W2pr_s = wtile("W2pr", W2pr_d, [HID, 4 * HID], F32R)
        b2T_s = wtile("b2T", b2T_d, [HID, 4], F32)
        Spa_s = wtile("Spa", Spa_d, [HID, LD], F32R)
        Spb_s = wtile("Spb", Spb_d, [HID, LD], F32R)
        encW_s = wtile("encW", encW_d, [65, LD], F32R)
        encb2_s = wtile("encb2", encb2_d, [LD, 1], F32)
        decW_s = wtile("decW", decW_d, [LD, SD], F32R)
        decb2_s = wtile("decb2", decb2_d, [SD, 1], F32)
        Gt_s = wtile("Gt", Gt_d, [L, NU * NI], F32R)
        idF_s = wtile("idF", idF_d, [L, L], F32)

        zp = st.enter_context(tc.tile_pool(name="zstate", bufs=2))

        obp = st.enter_context(tc.tile_pool(name="obuf", bufs=1))
        obuf = []
        for c in range(4):
            ob = obp.tile([L, L * SD], F32, tag=f"ob{c}", name=f"ob{c}")
            obuf.append(ob)

        otp = st.enter_context(tc.tile_pool(name="oT", bufs=2))

        def decode_z(zsrcs, col, pdopool, pdotag, tppool, tptag):
            """Decode group z tiles -> obuf column `col` (batch-major)."""
            pdo = pdopool.tile([SD, BC], F32, tag=pdotag, name="pdo")
            mmr(pdo[:, 0:BS], decW_s[:], zsrcs[0][:], start=True, stop=True)
            mmr(pdo[:, BS:BC], decW_s[:], zsrcs[1][:], start=True, stop=True)
            oT = otp.tile([SD, BC], F32, tag="oT")
            nc.scalar.activation(oT[:], pdo[:], AF.Identity, bias=decb2_s[:])
            for c in range(4):
                cs = slice(c * 128, (c + 1) * 128)
                ptp = tppool.tile([L, SD], F32, tag=tptag, name="ptp")
                nc.tensor.transpose(ptp[:], oT[:, cs], idF_s[0:SD, 0:SD])
                dst = obuf[c][:, col * SD:(col + 1) * SD]
                if c % 2 == 0:
                    nc.scalar.copy(dst, ptp[:])
                else:
                    nc.vector.tensor_scalar_add(dst, ptp[:], 0.0)

        # ---------- phases 1-3 ----------
        with tc.tile_pool(name="ph_sb", bufs=4) as php, \
             tc.tile_pool(name="ph_ps", bufs=2, space="PSUM") as ppp, \
             tc.tile_pool(name="ph_ps2", bufs=2, space="PSUM") as pp2:
            xT = php.tile([L, AD * BC], F32R, tag="xT")
            for cb in range(4):
                csl = slice(cb * 128, (cb + 1) * 128)
                ab = php.tile([128, L * AD], F16, tag="ab")
                nc.sync.dma_start(ab[:], a_d[csl, :])
                if _DBG and cb == 0:
                    nc.sync.dma_start(dbga_d, ab[:])
                abf = php.tile([128, L * AD], F32, tag="abf")
                nc.scalar.copy(abf[:], ab[:])
                a3 = abf[:].rearrange("b (l c) -> b l c", c=AD)
                for ch in range(AD):
                    pa = ppp.tile([L, 128], F32, tag="tp")
                    nc.tensor.transpose(pa[:], a3[:, :, ch], idF_s[:])
                    o = ch * BC + cb * 128
                    if ch % 2 == 0:
                        nc.scalar.copy(xT[:, o:o + 128], pa[:])
                    else:
                        nc.vector.tensor_scalar_add(xT[:, o:o + 128], pa[:], 0.0)

            # phase 2: dX for action channels (bf16 matmuls)
            for u in range(NU):
                for ch in range(AD):
                    pg = pp2.tile([NI, BC], F32, tag="g")
                    mmr(pg[:], Gt_s[:, u * NI:(u + 1) * NI],
                        xT[:, ch * BC:(ch + 1) * BC], start=True, stop=True)
                    gsb = php.tile([NI, BC], F32R, tag="gsb")
                    if ch % 2 == 0:
                        nc.scalar.copy(gsb[:], pg[:])
                    else:
                        nc.vector.tensor_scalar_add(gsb[:], pg[:], 0.0)
                    nc.sync.dma_start(dx_d[:, ch, u, :], gsb[:])

            # phase 3: encoder z0 (stacked)
            in0T = php.tile([65, BC], F32R, tag="in0T")
            nc.sync.dma_start(in0T[40:64, :], zpad_d)
            nc.sync.dma_start(in0T[0:SD, :], s0T_d)
            nc.sync.dma_start(in0T[SD:SD + AD, :], a0T_d)
            nc.sync.dma_start(in0T[64:65, :], t0_d)
            pzs = pp2.tile([LD, BC], F32, tag="g")
            mmr(pzs[:, 0:BS], encW_s[:], in0T[:, 0:BS], start=True, stop=True)
            mmr(pzs[:, BS:BC], encW_s[:], in0T[:, BS:BC], start=True,
                stop=True)
            zgs = []
            for g in range(2):
                zg = zp.tile([LD, BS], F32R, tag=f"z{g}", name=f"zg{g}")
                nc.scalar.activation(zg[:], pzs[:, g * BS:(g + 1) * BS],
                                     AF.Identity, bias=encb2_s[:])
                zgs.append(zg)
            nc.sync.dma_start(canary_d[:, 0:BS], zgs[0][:])
            nc.sync.dma_start(canary_d[:, BS:BC], zgs[1][:])
            if _DBG:
                nc.sync.dma_start(dbgz_d[:, 0:BS], zgs[0][:])
                nc.sync.dma_start(dbgin_d, in0T[:])
            decode_z(zgs, 0, ppp, "m", ppp, "tp")

        tc.strict_bb_all_engine_barrier()

        # ---------- scan pools (two independent batch groups) ----------
        pf = st.enter_context(tc.tile_pool(name="ps_f", bufs=4, space="PSUM"))
        ph2 = st.enter_context(tc.tile_pool(name="ps_h", bufs=2, space="PSUM"))
        pk = st.enter_context(tc.tile_pool(name="ps_k", bufs=2, space="PSUM"))
        hp = st.enter_context(tc.tile_pool(name="h_sb", bufs=3))
        fp_ = st.enter_context(tc.tile_pool(name="f_sb", bufs=3))
        pp = st.enter_context(tc.tile_pool(name="p_sb", bufs=2))
        ztp = st.enter_context(tc.tile_pool(name="zt_sb", bufs=2))
        wwp = st.enter_context(tc.tile_pool(name="w_sb", bufs=2))
        bcp = st.enter_context(tc.tile_pool(name="bc_sb", bufs=2))

        zcur = zgs
        for i in range(NI):
            bt = bcp.tile([HID, NU * 4 * BC], F32R, tag="bc")
            for u in range(NU):
                lo = dx_d[i, 0, u, :]
                hi = dx_d[i, 1, u, :]
                csl = slice(u * 4 * BC, (u + 1) * 4 * BC)
                nc.sync.dma_start(
                    bt[0:64, csl],
                    bass.AP(lo.tensor, lo.offset,
                            [[0, 64], [2 * NU * BC, 4], [1, BC]]))
                nc.sync.dma_start(
                    bt[64:128, csl],
                    bass.AP(hi.tensor, hi.offset,
                            [[0, 64], [2 * NU * BC, 4], [1, BC]]))

            for sub in range(2):
                uix = [0, 1, 1, 2] if sub == 0 else [2, 3, 3, 4]
                kacc = [pk.tile([LD, BS], F32, tag="k", name=f"kacc{g}")
                        for g in range(2)]
                ztmp = list(zcur)
                w23 = [None, None]
                znew = [None, None]
                for s in range(4):
                    Sp_s = Spa_s if s in (0, 3) else Spb_s
                    W2x0_s = W2x0a_s if s in (0, 3) else W2x0b_s
                    st_flag = (s == 0)
                    u4 = uix[s] * 4

                    hsbs = []
                    for g in range(2):
                        hps = ph2.tile([HID, BS], F32, tag="h",
                                       name=f"hps{g}")
                        mmr(hps[:], W1_s[:], ztmp[g][:], start=True,
                            stop=True)
                        hsbs.append(hps)
                    hsbt = []
                    for g in range(2):
                        hsb = hp.tile([HID, BS], F32R, tag=f"h{g}",
                                      name=f"hsb{g}")
                        nc.scalar.activation(hsb[:], hsbs[g][:], AF.Relu,
                                             bias=b1_s[:])
                        hsbt.append(hsb)

                    fpss = [[None] * 4, [None] * 4]
                    for pt in (1, 2, 0, 3):
                        for g in range(2):
                            fps = pf.tile([HID, BS], F32, tag="f",
                                          name=f"fps{g}_{pt}")
                            mmr(fps[:], W2pr_s[:, pt * 128:(pt + 1) * 128],
                                hsbt[g][:], start=True, stop=True)
                            fpss[g][pt] = fps

                    # mults: pt1/pt2 via ACT copy + Pool; pt0/pt3 DVE STT
                    psbs = [[None] * 4, [None] * 4]
                    for g in range(2):
                        for pt in (1, 2):
                            fsb = fp_.tile([HID, BS], F32R, tag=f"fc{g}",
                                           name=f"fsb{g}_{pt}")
                            nc.scalar.activation(fsb[:], fpss[g][pt][:],
                                                 AF.Identity,
                                                 bias=b2T_s[:, pt:pt + 1])
                            psb = pp.tile([HID, BS], F32R, tag=f"pp{g}_{pt}",
                                          name=f"psbp{g}_{pt}")
                            bsl = bt[:, (u4 + pt) * BC + g * BS:
                                     (u4 + pt) * BC + g * BS + BS]
                            nc.gpsimd.tensor_tensor(psb[:], fsb[:], bsl,
                                                    op=ALU.mult)
                            psbs[g][pt] = psb
                    for g in range(2):
                        for pt in (0, 3):
                            psb = pp.tile([HID, BS], F32R, tag=f"pv{g}_{pt}",
                                          name=f"psbv{g}_{pt}")
                            bsl = bt[:, (u4 + pt) * BC + g * BS:
                                     (u4 + pt) * BC + g * BS + BS]
                            STT(psb[:], fpss[g][pt][:], b2T_s[:, pt:pt + 1],
                                bsl, op0=ALU.add, op1=ALU.mult)
                            psbs[g][pt] = psb

                    for g in range(2):
                        mmr(kacc[g][:], W2x0_s[:], hsbt[g][:],
                            start=st_flag, stop=False, skip_group_check=True)
                    for j, pt in enumerate((0, 3, 1, 2)):
                        for g in range(2):
                            mmr(kacc[g][:], Sp_s[:], psbs[g][pt][:],
                                start=False, stop=(s == 3 and j == 3),
                                skip_group_check=True)

                    for g in range(2):
                        if s == 0:
                            zt = ztp.tile([LD, BS], F32R, tag=f"zt{g}",
                                          name=f"zt{g}")
                            STT(zt[:], kacc[g][:], 0.25, zcur[g][:],
                                op0=ALU.mult, op1=ALU.add)
                            ztmp[g] = zt
                            ww = wwp.tile([LD, BS], F32, tag=f"w{g}",
                                          name=f"ww{g}")
                            STT(ww[:], kacc[g][:], -0.125, zcur[g][:],
                                op0=ALU.mult, op1=ALU.add)
                            w23[g] = ww
                        elif s == 1:
                            zt = ztp.tile([LD, BS], F32R, tag=f"zt{g}",
                                          name=f"zt2{g}")
                            STT(zt[:], kacc[g][:], 0.125, w23[g][:],
                                op0=ALU.mult, op1=ALU.add)
                            ztmp[g] = zt
                            ww = wwp.tile([LD, BS], F32, tag=f"w{g}",
                                          name=f"ww2{g}")
                            STT(ww[:], kacc[g][:], -0.25, zcur[g][:],
                                op0=ALU.mult, op1=ALU.add)
                            w23[g] = ww
                        elif s == 2:
                            zt = ztp.tile([LD, BS], F32R, tag=f"zt{g}",
                                          name=f"zt3{g}")
                            STT(zt[:], kacc[g][:], 0.25, w23[g][:],
                                op0=ALU.mult, op1=ALU.add)
                            ztmp[g] = zt
                        else:
                            zn = zp.tile([LD, BS], F32R, tag=f"z{g}",
                                         name=f"zn{g}")
                            STT(zn[:], kacc[g][:], 1.0 / 12.0, zcur[g][:],
                                op0=ALU.mult, op1=ALU.add)
                            znew[g] = zn
                zcur = znew

            decode_z(zcur, i + 1, pk, "k", pf, "f")

        # final output: convert to bf16, DMA per chunk
        with tc.tile_pool(name="ob16", bufs=1) as obq:
            for c in range(4):
                ob16 = obq.tile([L, L * SD], F16, tag="ob16")
                nc.scalar.copy(ob16[:], obuf[c][:])
                nc.sync.dma_start(out_ds[c], ob16[:])

    nc.compile()
    return nc


def _get_program():
    if "nc" not in _CACHE:
        _CACHE["nc"] = _build_program()
    return _CACHE["nc"]


def build_in_maps(s, a, t, enc_W, enc_b, f_W1, f_b1, f_W2, f_b2, dec_W, dec_b):
    import ml_dtypes

    bf16 = ml_dtypes.bfloat16

    if "consts" not in _CACHE:
        Gt = _host_consts()
        W2x0, W2pr, b2T, _b2x0 = _perm_w2(f_W2, f_b2)
        I64 = np.eye(LD, dtype=np.float32)
        Spair = np.concatenate([I64, I64], axis=0)     # [128, 64]
        encb = np.asarray(enc_b, np.float32).reshape(LD)
        decb = np.asarray(dec_b, np.float32).reshape(SD)
        ident = np.eye(L, dtype=np.float32)
        _CACHE["consts"] = dict(
            W1=np.ascontiguousarray(np.asarray(f_W1, np.float32)),
            zpad=np.zeros((24, BC), np.float32),
            b1=np.asarray(f_b1, np.float32).reshape(HID, 1).copy(),
            W2x0a=W2x0, W2x0b=np.ascontiguousarray(2.0 * W2x0),
            W2pr=W2pr, b2T=b2T,
            Spa=Spair, Spb=np.ascontiguousarray(2.0 * Spair),
            encW=_pad_encw(enc_W),
            encb2=encb.reshape(LD, 1).copy(),
            decW=np.ascontiguousarray(np.asarray(dec_W, np.float32)),
            decb2=decb.reshape(SD, 1).copy(),
            Gt=Gt,
            idF=np.eye(L, dtype=np.float32),
        )
    const_map = _CACHE["consts"]

    s = np.asarray(s, np.float32)
    a = np.asarray(a, np.float32)
    t = np.asarray(t, np.float32)
    a16 = a.astype(np.float16)

    in_maps = []
    for c in range(NCORES):
        rs = slice(c * BC, (c + 1) * BC)
        m = dict(const_map)
        m["s0T"] = np.ascontiguousarray(s[rs, 0, :].T)
        m["a0T"] = np.ascontiguousarray(a[rs, 0, :].T)
        m["t0"] = np.ascontiguousarray(t[rs, 0].reshape(1, BC))
        m["a_in"] = np.ascontiguousarray(a16[rs].reshape(BC, L * AD))
        in_maps.append(m)
    return in_maps


def kernel(s, a, t, enc_W, enc_b, f_W1, f_b1, f_W2, f_b2, dec_W, dec_b):
    global LAST_RESULTS
    from concourse.bass_utils import run_bass_kernel_spmd

    in_maps = build_in_maps(s, a, t, enc_W, enc_b, f_W1, f_b1, f_W2, f_b2,
                            dec_W, dec_b)
    nc = _get_program()
    res = run_bass_kernel_spmd(nc, in_maps, core_ids=list(range(NCORES)))
    LAST_RESULTS = res

    out = np.empty((B, L, SD), np.float32)
    for c in range(NCORES):
        for ch in range(4):
            o = np.asarray(res.results[c][f"out{ch}"], np.float32)
            out[c * BC + ch * 128:c * BC + (ch + 1) * 128] = \
                o.reshape(128, L, SD)
    return out
